# revision 17
# baseline (speedup 1.0000x reference)
"""Trainium2 Bass kernel for causal MHA block (b=4, s=2048, E=1024, 16 heads).

Sharding: tensor-parallel over heads — 2 heads per core across 8 cores.
Each core computes Q^T/K^T (transposed layout, head-packed), V (natural
layout, ones-augmented), block-causal attention with softmax denominators
obtained for free from the ones column, and a partial out-projection over
its 128 embedding dims. Host sums the 8 partials and adds out_b.

v3 scheduling (the math is the baseline's; the issue order isn't):
  - software-pipelined attention: scores+exp for key-tile i+1 issue before
    the ctx matmuls of tile i, so PE never blocks on the ACT round trip.
  - proj/out-proj matmuls interleave as fillers BETWEEN attention
    iterations (deadline-JIT: proj chunk 4b+j+1 runs inside attn chunk
    (b,j)), keeping PE continuously busy (no HAM re-throttle).
  - key-padding mask folded into the exp bias (per-partition = per-key
    0/-10000), so V needs no masking and the transpose drain is a plain
    copy; the ones-column denominator stays exact (padded probs are 0).
  - per-head 1-bank score tiles, bufs=3: deeper rotation than one 2-bank
    tile pair, and frees a PSUM bank for the work pool.
  - PSUM: scores [128,512]x3 + ctx [65,512]x2 + work [128,512]x3 = 8 banks.
  - all PSUM->SBUF staging on DVE; exp exclusively on ACT; filler order
    keeps work-pool allocations >= 3 apart in time.
"""

import sys
from contextlib import ExitStack

import numpy as np

sys.path.insert(0, "/opt/trn_rl_repo")

import concourse.bass as bass  # noqa: E402
import concourse.tile as tile  # noqa: E402
from concourse import bacc  # noqa: E402
from concourse import mybir  # noqa: E402

F32 = mybir.dt.float32
BF16 = mybir.dt.bfloat16
AF = mybir.ActivationFunctionType

NEG = -10000.0
N_CORES = 8


def build_program(B=4, S=2048, io_dt=BF16):
    P = 128
    E = 1024
    ET = E // P            # 8 E-tiles
    RC = 512               # row chunk for projections
    NCH = S // RC          # proj chunks per batch (4)
    NT = S // P            # s-tiles per batch (16)
    TJ = S // 512          # query chunks of 512 per batch (4)
    ROWS = B * S

    nc = bacc.Bacc("TRN2", target_bir_lowering=False, debug=False)

    xT_d = nc.declare_dram_parameter("xT", [E, ROWS], io_dt, isOutput=False)
    wq_d = nc.declare_dram_parameter("wq", [E, P], io_dt, isOutput=False)
    wk_d = nc.declare_dram_parameter("wk", [E, P], io_dt, isOutput=False)
    wv_d = nc.declare_dram_parameter("wv", [E, P], io_dt, isOutput=False)
    bq_d = nc.declare_dram_parameter("bq", [P, 1], F32, isOutput=False)
    bv_d = nc.declare_dram_parameter("bv", [P, 1], F32, isOutput=False)
    ow_d = nc.declare_dram_parameter("ow", [P, E], io_dt, isOutput=False)
    mskb_d = nc.declare_dram_parameter("mskb", [P, B * NT], F32, isOutput=False)
    tri_d = nc.declare_dram_parameter("tri", [P, P], io_dt, isOutput=False)
    idn_d = nc.declare_dram_parameter("idn", [P, P], io_dt, isOutput=False)
    out_d = nc.declare_dram_parameter("outp", [ROWS, E], io_dt, isOutput=True)

    with ExitStack() as ctx:
        tc = ctx.enter_context(tile.TileContext(nc))
        const = ctx.enter_context(tc.tile_pool(name="const", bufs=1))

        wq_sb = [const.tile([P, P], io_dt, tag=f"wq{et}", name=f"wq{et}")
                 for et in range(ET)]
        wk_sb = [const.tile([P, P], io_dt, tag=f"wk{et}", name=f"wk{et}")
                 for et in range(ET)]
        wv_sb = [const.tile([P, P], io_dt, tag=f"wv{et}", name=f"wv{et}")
                 for et in range(ET)]
        for et in range(ET):
            esl = slice(et * P, (et + 1) * P)
            nc.gpsimd.dma_start(wq_sb[et][:], wq_d[esl, :])
            nc.scalar.dma_start(wk_sb[et][:], wk_d[esl, :])
            (nc.gpsimd if et % 2 else nc.scalar).dma_start(wv_sb[et][:], wv_d[esl, :])
        ow_sb = const.tile([P, E], io_dt, tag="ow")
        nc.scalar.dma_start(ow_sb[:], ow_d[:])
        bq_sb = const.tile([P, 1], F32, tag="bq")
        nc.gpsimd.dma_start(bq_sb[:], bq_d[:])
        bv_sb = const.tile([P, 1], F32, tag="bv")
        nc.gpsimd.dma_start(bv_sb[:], bv_d[:])
        mskb_sb = const.tile([P, B * NT], F32, tag="mskb")
        nc.gpsimd.dma_start(mskb_sb[:], mskb_d[:])
        tri_sb = const.tile([P, P], io_dt, tag="tri")
        nc.gpsimd.dma_start(tri_sb[:], tri_d[:])
        idn_sb = const.tile([P, P], io_dt, tag="idn")
        nc.gpsimd.dma_start(idn_sb[:], idn_d[:])
        ones_sb = const.tile([1, P], io_dt, tag="ones")
        nc.any.memset(ones_sb[:], 1.0)

        # per-batch projection outputs: q/k packed in one tile
        qkt_sbs = [const.tile([P, 2, S], io_dt, tag=f"qk{b}", name=f"qk{b}")
                   for b in range(B)]
        v_sbs = [const.tile([P, NT, 2, 65], io_dt, tag=f"v{b}", name=f"v{b}")
                 for b in range(B)]
        # ones-augmentation columns: plain 1.0 (padded keys are zeroed via
        # the exp bias, so probs — and hence the denominator — are exact)
        for b in range(B):
            for h in range(2):
                nc.vector.memset(v_sbs[b][:, :, h, 64:65], 1.0)

        xpool = ctx.enter_context(tc.tile_pool(name="xp", bufs=2))
        ppool = ctx.enter_context(tc.tile_pool(name="pt", bufs=6))
        cpool = ctx.enter_context(tc.tile_pool(name="cn", bufs=2))
        spool = ctx.enter_context(tc.tile_pool(name="sm", bufs=2))
        opool = ctx.enter_context(tc.tile_pool(name="ot", bufs=4))
        # PSUM: "s" scores [128,512] x3; "c" ctx accum x2; "w" work x3 = 8
        psS = ctx.enter_context(tc.tile_pool(name="psS", bufs=3, space="PSUM"))
        psC = ctx.enter_context(tc.tile_pool(name="psC", bufs=2, space="PSUM"))
        psW = ctx.enter_context(tc.tile_pool(name="psW", bufs=3, space="PSUM"))

        def emit_x_dma(pc):
            """Prefetch x tiles for proj chunk pc (global index)."""
            b, ch = divmod(pc, NCH)
            r0 = b * S + ch * RC
            xt = xpool.tile([P, ET, RC], io_dt, tag="xt", name="xt")
            for et in range(ET):
                nc.sync.dma_start(xt[:, et], xT_d[et * P:(et + 1) * P, r0:r0 + RC])
            return xt

        def proj_accum_units(pc, xt, w):
            """Filler closures for one projection (w: 0=q 1=k 2=v) of proj
            chunk pc — a single work-pool accumulation over 8 matmuls."""
            b, ch = divmod(pc, NCH)
            rsb = slice(ch * RC, (ch + 1) * RC)
            acc = [None]

            def p_mm(et):
                if et == 0:
                    acc[0] = psW.tile([P, RC], F32, tag="w", name="pps")
                w_sb = (wq_sb, wk_sb, wv_sb)[w]
                nc.tensor.matmul(
                    acc[0][:], w_sb[et][:], xt[:, et],
                    start=(et == 0), stop=(et == ET - 1),
                )
                if et == ET - 1:
                    if w == 0:
                        nc.vector.tensor_scalar_add(
                            qkt_sbs[b][:, 0, rsb], acc[0][:], bq_sb[:])
                    elif w == 1:
                        # K bias cancels in softmax; plain downcast copy
                        nc.vector.tensor_copy(qkt_sbs[b][:, 1, rsb], acc[0][:])
                    else:
                        vt = xpool.tile([P, RC], io_dt, tag="vt", name="vt")
                        nc.vector.tensor_scalar_add(
                            vt[:], acc[0][:], bv_sb[:])
                        vts[pc] = vt

            for et in range(ET):
                yield lambda et=et: p_mm(et)

        vts = {}

        def t_units(pc):
            """V-transpose fillers (one per 128-key tile of proj chunk pc)."""
            b, ch = divmod(pc, NCH)

            def t_mm(rt4):
                rt = ch * (RC // P) + rt4
                trp = psW.tile([P, 2, 64], io_dt, tag="w", name="trp")
                nc.tensor.transpose(
                    trp[:], vts[pc][:, rt4 * P:(rt4 + 1) * P], idn_sb[:])
                nc.vector.tensor_copy(v_sbs[b][:, rt, :, 0:64], trp[:])

            for rt4 in range(RC // P):
                yield lambda rt4=rt4: t_mm(rt4)

        def outproj_units(b, j, cn):
            """Out-projection fillers for chunk (b,j)."""
            t0 = j * 512

            def op_mm(rt4, fc):
                r0 = b * S + t0 + rt4 * P
                ops = psW.tile([P, 512], F32, tag="w", name="ops")
                nc.tensor.matmul(
                    ops[:],
                    cn[:, rt4 * P:(rt4 + 1) * P],
                    ow_sb[:, fc * 512:(fc + 1) * 512],
                    start=True, stop=True,
                )
                ot = opool.tile([P, 512], io_dt, tag="ot", name="ot")
                nc.vector.tensor_copy(ot[:], ops[:])
                nc.sync.dma_start(
                    out_d[r0:r0 + P, fc * 512:(fc + 1) * 512], ot[:])

            for rt4 in range(4):
                for fc in range(2):
                    yield lambda rt4=rt4, fc=fc: op_mm(rt4, fc)

        def chunk_fillers(pc, ops):
            """Interleave-friendly filler order: out-proj units separate the
            three proj accumulations and the transposes, so consecutive
            work-pool allocations stay well apart in time."""
            units = []
            o = list(ops)
            if pc is not None:
                xt = xts.pop(pc)
                for w in range(3):
                    if o:
                        units.append(o.pop(0))
                    units.extend(proj_accum_units(pc, xt, w))
                for t in t_units(pc):
                    if o:
                        units.append(o.pop(0))
                    units.append(t)
            units.extend(o)
            return units

        def attn_chunk(b, j, fillers):
            """Attention for query chunk (b,j) with fillers interleaved."""
            t0 = j * 512
            nv = 4 * j + 4
            cn = cpool.tile([P, 512], io_dt, tag="cn", name="cn")
            cps = [psC.tile([65, 512], F32, tag="c", name=f"cps{h}")
                   for h in range(2)]
            nf = len(fillers)
            fi = 0
            resv = min(6, nf)
            navail = nf - resv
            pend = [None]

            def issue_scores(i):
                delta = i * P - t0
                col0 = max(0, delta)
                sg = i * P
                bias = mskb_sb[:, b * NT + i:b * NT + i + 1]
                pts = []
                for h in range(2):
                    hp = slice(h * 64, (h + 1) * 64)
                    sp = psS.tile([P, 512], F32, tag="s", name="sp")
                    nc.tensor.matmul(
                        sp[:, col0:512],
                        qkt_sbs[b][hp, 1, sg:sg + P],
                        qkt_sbs[b][hp, 0, t0 + col0:t0 + 512],
                        start=True, stop=(delta < 0),
                    )
                    if delta >= 0:  # diagonal: add causal triangle on PE
                        nc.tensor.matmul(
                            sp[:, col0:col0 + P], idn_sb[:], tri_sb[:],
                            start=False, stop=True,
                        )
                    pt = ppool.tile([P, 512], io_dt, tag="pt", name="pt")
                    nc.scalar.activation(
                        pt[:, col0:512], sp[:, col0:512], AF.Exp, bias=bias)
                    pts.append(pt)
                return (pts, col0, i)

            def issue_ctx(pts, col0, i):
                for h in range(2):
                    nc.tensor.matmul(
                        cps[h][:, col0:512],
                        v_sbs[b][:, i, h],
                        pts[h][:, col0:512],
                        start=(i == 0), stop=(i == nv - 1),
                    )

            for i in range(nv):
                hi = (navail * (i + 1)) // nv
                if fi < hi:
                    fillers[fi]()
                    fi += 1
                nxt = issue_scores(i)
                while fi < hi:
                    fillers[fi]()
                    fi += 1
                if pend[0] is not None:
                    issue_ctx(*pend[0])
                pend[0] = nxt
            issue_ctx(*pend[0])

            # normalization: denom rows -> per-head PE broadcast into one
            # work tile -> one reciprocal -> per-head scale.  Reserved
            # fillers cover the DVE->PE round trips.
            dens = []
            for h in range(2):
                den = spool.tile([1, 512], io_dt, tag="den", name="den")
                nc.vector.tensor_copy(den[:], cps[h][64:65, :])
                dens.append(den)
            while fi < nf - 2:
                fillers[fi]()
                fi += 1
            bps = psW.tile([P, 512], F32, tag="w", name="bps")
            for h in range(2):
                hp = slice(h * 64, (h + 1) * 64)
                nc.tensor.matmul(bps[hp, :], ones_sb[:, 0:64], dens[h][:],
                                 start=True, stop=True)
            while fi < nf:
                fillers[fi]()
                fi += 1
            rc = spool.tile([P, 512], F32, tag="rc", name="rc")
            nc.vector.reciprocal_approx_fast(rc[:], bps[:])
            for h in range(2):
                hp = slice(h * 64, (h + 1) * 64)
                nc.vector.tensor_mul(cn[hp, :], cps[h][0:64, :], rc[hp, :])
            return cn

        # ---- emission schedule ----
        xts = {0: emit_x_dma(0), 1: emit_x_dma(1)}
        for u in chunk_fillers(0, []):
            u()
        prev = None  # (b, j, cn) of the chunk awaiting out-projection
        for b in range(B):
            for j in range(TJ):
                pc = 4 * b + j + 1
                if pc + 1 < B * NCH:
                    xts[pc + 1] = emit_x_dma(pc + 1)
                ops = list(outproj_units(*prev)) if prev is not None else []
                fillers = chunk_fillers(pc if pc < B * NCH else None, ops)
                cn = attn_chunk(b, j, fillers)
                prev = (b, j, cn)
        for u in outproj_units(*prev):
            u()
    nc.compile()
    return nc


def make_core_inputs(x, key_padding_mask, Wqkv_w, Wqkv_b, out_w, B=4, S=2048,
                     np_io=None):
    """Host-side shard prep. Returns list of in_maps per core."""
    import ml_dtypes
    if np_io is None:
        np_io = ml_dtypes.bfloat16
    E = 1024
    P = 128
    NT = S // P
    x = np.asarray(x, np.float32)
    mask = np.asarray(key_padding_mask)
    Wqkv_w = np.asarray(Wqkv_w, np.float32)
    Wqkv_b = np.asarray(Wqkv_b, np.float32)
    out_w = np.asarray(out_w, np.float32)

    xT = np.ascontiguousarray(x.reshape(B * S, E).T).astype(np_io)
    m01 = mask.astype(np.float32)  # 1 valid / 0 padded
    mskb = np.where(m01 > 0.5, 0.0, NEG).astype(np.float32)
    mskb_t = np.ascontiguousarray(mskb.reshape(B * NT, P).T)  # [128, B*NT]
    r = np.arange(P)
    tri = np.where(r[:, None] > r[None, :], NEG, 0.0).astype(np_io)
    idn = np.eye(P, dtype=np.float32).astype(np_io)
    scale = 1.0 / np.sqrt(64.0)

    in_maps = []
    for c in range(N_CORES):
        hA, hB = 2 * c, 2 * c + 1
        sel = np.r_[hA * 64:(hA + 1) * 64, hB * 64:(hB + 1) * 64]
        wq = np.ascontiguousarray(Wqkv_w[sel].T).astype(np_io)
        wk = np.ascontiguousarray((Wqkv_w[E + sel] * scale).T).astype(np_io)
        wv = np.ascontiguousarray(Wqkv_w[2 * E + sel].T).astype(np_io)
        bq = np.ascontiguousarray(Wqkv_b[sel][:, None]).astype(np.float32)
        bv = np.ascontiguousarray(Wqkv_b[2 * E + sel][:, None]).astype(np.float32)
        ow = np.ascontiguousarray(out_w[:, sel].T).astype(np_io)
        in_maps.append({
            "xT": xT, "wq": wq, "wk": wk, "wv": wv,
            "bq": bq, "bv": bv, "ow": ow, "mskb": mskb_t,
            "tri": tri, "idn": idn,
        })
    return in_maps


_NC_CACHE = {}


def _get_nc(B=4, S=2048, io_dt=BF16):
    key = (B, S, io_dt)
    if key not in _NC_CACHE:
        _NC_CACHE[key] = build_program(B, S, io_dt)
    return _NC_CACHE[key]


def run_full(inputs, trace=False, tmpdir=None, io_dt=BF16, np_io=None):
    from concourse.bass_utils import run_bass_kernel_spmd

    B, S, E = 4, 2048, 1024
    nc = _get_nc(B, S, io_dt)
    in_maps = make_core_inputs(
        inputs["x"], inputs["key_padding_mask"], inputs["Wqkv_w"],
        inputs["Wqkv_b"], inputs["out_w"], B, S, np_io=np_io,
    )
    res = run_bass_kernel_spmd(
        nc, in_maps, list(range(N_CORES)), trace=trace, tmpdir=tmpdir,
    )
    acc = res.results[0]["outp"].astype(np.float32)
    for c in range(1, N_CORES):
        acc = acc + res.results[c]["outp"].astype(np.float32)
    out = acc + np.asarray(inputs["out_b"], np.float32)[None, :]
    return out.reshape(B, S, E), res


def kernel(**inputs) -> np.ndarray:
    out, _ = run_full(inputs)
    return out


# revision 22
# speedup vs baseline: 1.0049x; 1.0049x over previous
"""Trainium2 Bass kernel for causal MHA block (b=4, s=2048, E=1024, 16 heads).

Sharding: tensor-parallel over heads — 2 heads per core across 8 cores.
Each core computes Q^T/K^T (transposed layout, head-packed), V (natural
layout, ones-augmented), block-causal attention with softmax denominators
obtained for free from the ones column, and a partial out-projection over
its 128 embedding dims. Host sums the 8 partials and adds out_b.

v3 scheduling (the math is the baseline's; the issue order isn't):
  - software-pipelined attention: scores+exp for key-tile i+1 issue before
    the ctx matmuls of tile i, so PE never blocks on the ACT round trip.
  - proj/out-proj matmuls interleave as fillers BETWEEN attention
    iterations (deadline-JIT: proj chunk 4b+j+1 runs inside attn chunk
    (b,j)), keeping PE continuously busy (no HAM re-throttle).
  - key-padding mask folded into the exp bias (per-partition = per-key
    0/-10000), so V needs no masking and the transpose drain is a plain
    copy; the ones-column denominator stays exact (padded probs are 0).
  - per-head 1-bank score tiles, bufs=3: deeper rotation than one 2-bank
    tile pair, and frees a PSUM bank for the work pool.
  - PSUM: scores [128,512]x3 + ctx [65,512]x2 + work [128,512]x3 = 8 banks.
  - all PSUM->SBUF staging on DVE; exp exclusively on ACT; filler order
    keeps work-pool allocations >= 3 apart in time.
"""

import sys
from contextlib import ExitStack

import numpy as np

sys.path.insert(0, "/opt/trn_rl_repo")

import concourse.bass as bass  # noqa: E402
import concourse.tile as tile  # noqa: E402
from concourse import bacc  # noqa: E402
from concourse import mybir  # noqa: E402

F32 = mybir.dt.float32
BF16 = mybir.dt.bfloat16
AF = mybir.ActivationFunctionType

NEG = -10000.0
N_CORES = 8


def build_program(B=4, S=2048, io_dt=BF16):
    P = 128
    E = 1024
    ET = E // P            # 8 E-tiles
    RC = 512               # row chunk for projections
    NCH = S // RC          # proj chunks per batch (4)
    NT = S // P            # s-tiles per batch (16)
    TJ = S // 512          # query chunks of 512 per batch (4)
    ROWS = B * S

    nc = bacc.Bacc("TRN2", target_bir_lowering=False, debug=False)

    xT_d = nc.declare_dram_parameter("xT", [E, ROWS], io_dt, isOutput=False)
    wq_d = nc.declare_dram_parameter("wq", [E, P], io_dt, isOutput=False)
    wk_d = nc.declare_dram_parameter("wk", [E, P], io_dt, isOutput=False)
    wv_d = nc.declare_dram_parameter("wv", [E, P], io_dt, isOutput=False)
    bq_d = nc.declare_dram_parameter("bq", [P, 1], F32, isOutput=False)
    bv_d = nc.declare_dram_parameter("bv", [P, 1], F32, isOutput=False)
    ow_d = nc.declare_dram_parameter("ow", [P, E], io_dt, isOutput=False)
    mskb_d = nc.declare_dram_parameter("mskb", [P, B * NT], F32, isOutput=False)
    tri_d = nc.declare_dram_parameter("tri01", [P, P], io_dt, isOutput=False)
    idn_d = nc.declare_dram_parameter("idn", [P, P], io_dt, isOutput=False)
    out_d = nc.declare_dram_parameter("outp", [ROWS, E], io_dt, isOutput=True)

    with ExitStack() as ctx:
        tc = ctx.enter_context(tile.TileContext(nc))
        const = ctx.enter_context(tc.tile_pool(name="const", bufs=1))

        wq_sb = [const.tile([P, P], io_dt, tag=f"wq{et}", name=f"wq{et}")
                 for et in range(ET)]
        wk_sb = [const.tile([P, P], io_dt, tag=f"wk{et}", name=f"wk{et}")
                 for et in range(ET)]
        wv_sb = [const.tile([P, P], io_dt, tag=f"wv{et}", name=f"wv{et}")
                 for et in range(ET)]
        for et in range(ET):
            esl = slice(et * P, (et + 1) * P)
            nc.gpsimd.dma_start(wq_sb[et][:], wq_d[esl, :])
            nc.scalar.dma_start(wk_sb[et][:], wk_d[esl, :])
            (nc.gpsimd if et % 2 else nc.scalar).dma_start(wv_sb[et][:], wv_d[esl, :])
        ow_sb = const.tile([P, E], io_dt, tag="ow")
        nc.scalar.dma_start(ow_sb[:], ow_d[:])
        bq_sb = const.tile([P, 1], F32, tag="bq")
        nc.gpsimd.dma_start(bq_sb[:], bq_d[:])
        bv_sb = const.tile([P, 1], F32, tag="bv")
        nc.gpsimd.dma_start(bv_sb[:], bv_d[:])
        mskb_sb = const.tile([P, B * NT], F32, tag="mskb")
        nc.gpsimd.dma_start(mskb_sb[:], mskb_d[:])
        tri_sb = const.tile([P, P], io_dt, tag="tri")
        nc.gpsimd.dma_start(tri_sb[:], tri_d[:])
        idn_sb = const.tile([P, P], io_dt, tag="idn")
        nc.gpsimd.dma_start(idn_sb[:], idn_d[:])
        ones_sb = const.tile([1, P], io_dt, tag="ones")
        nc.any.memset(ones_sb[:], 1.0)

        # per-batch projection outputs: q/k packed in one tile
        qkt_sbs = [const.tile([P, 2, S], io_dt, tag=f"qk{b}", name=f"qk{b}")
                   for b in range(B)]
        v_sbs = [const.tile([P, NT, 2, 65], io_dt, tag=f"v{b}", name=f"v{b}")
                 for b in range(B)]
        # ones-augmentation columns: plain 1.0 (padded keys are zeroed via
        # the exp bias, so probs — and hence the denominator — are exact)
        for b in range(B):
            for h in range(2):
                nc.vector.memset(v_sbs[b][:, :, h, 64:65], 1.0)

        xpool = ctx.enter_context(tc.tile_pool(name="xp", bufs=2))
        ppool = ctx.enter_context(tc.tile_pool(name="pt", bufs=6))
        cpool = ctx.enter_context(tc.tile_pool(name="cn", bufs=2))
        spool = ctx.enter_context(tc.tile_pool(name="sm", bufs=2))
        opool = ctx.enter_context(tc.tile_pool(name="ot", bufs=4))
        # PSUM: "s" scores [128,512] x3; "c" ctx accum x2; "w" work x3 = 8
        psS = ctx.enter_context(tc.tile_pool(name="psS", bufs=3, space="PSUM"))
        psC = ctx.enter_context(tc.tile_pool(name="psC", bufs=2, space="PSUM"))
        psW = ctx.enter_context(tc.tile_pool(name="psW", bufs=3, space="PSUM"))

        def emit_x_dma(pc):
            """Prefetch x tiles for proj chunk pc (global index)."""
            b, ch = divmod(pc, NCH)
            r0 = b * S + ch * RC
            xt = xpool.tile([P, ET, RC], io_dt, tag="xt", name="xt")
            for et in range(ET):
                nc.sync.dma_start(xt[:, et], xT_d[et * P:(et + 1) * P, r0:r0 + RC])
            return xt

        def proj_accum_units(pc, xt, w):
            """Filler closures for one projection (w: 0=q 1=k 2=v) of proj
            chunk pc — a single work-pool accumulation over 8 matmuls."""
            b, ch = divmod(pc, NCH)
            rsb = slice(ch * RC, (ch + 1) * RC)
            acc = [None]

            def p_mm(et):
                if et == 0:
                    acc[0] = psW.tile([P, RC], F32, tag="w", name="pps")
                w_sb = (wq_sb, wk_sb, wv_sb)[w]
                nc.tensor.matmul(
                    acc[0][:], w_sb[et][:], xt[:, et],
                    start=(et == 0), stop=(et == ET - 1),
                )
                if et == ET - 1:
                    if w == 0:
                        nc.vector.tensor_scalar_add(
                            qkt_sbs[b][:, 0, rsb], acc[0][:], bq_sb[:])
                    elif w == 1:
                        # K bias cancels in softmax; plain downcast copy
                        nc.vector.tensor_copy(qkt_sbs[b][:, 1, rsb], acc[0][:])
                    else:
                        vt = xpool.tile([P, RC], io_dt, tag="vt", name="vt")
                        nc.vector.tensor_scalar_add(
                            vt[:], acc[0][:], bv_sb[:])
                        vts[pc] = vt

            for et in range(ET):
                yield lambda et=et: p_mm(et)

        vts = {}

        def t_units(pc):
            """V-transpose fillers (one per 128-key tile of proj chunk pc)."""
            b, ch = divmod(pc, NCH)

            def t_mm(rt4):
                rt = ch * (RC // P) + rt4
                trp = psW.tile([P, 2, 64], io_dt, tag="w", name="trp")
                nc.tensor.transpose(
                    trp[:], vts[pc][:, rt4 * P:(rt4 + 1) * P], idn_sb[:])
                nc.vector.tensor_copy(v_sbs[b][:, rt, :, 0:64], trp[:])

            for rt4 in range(RC // P):
                yield lambda rt4=rt4: t_mm(rt4)

        def outproj_units(b, j, cn):
            """Out-projection fillers for chunk (b,j)."""
            t0 = j * 512

            def op_mm(rt4, fc):
                r0 = b * S + t0 + rt4 * P
                ops = psW.tile([P, 512], F32, tag="w", name="ops")
                nc.tensor.matmul(
                    ops[:],
                    cn[:, rt4 * P:(rt4 + 1) * P],
                    ow_sb[:, fc * 512:(fc + 1) * 512],
                    start=True, stop=True,
                )
                ot = opool.tile([P, 512], io_dt, tag="ot", name="ot")
                nc.vector.tensor_copy(ot[:], ops[:])
                nc.sync.dma_start(
                    out_d[r0:r0 + P, fc * 512:(fc + 1) * 512], ot[:])

            for rt4 in range(4):
                for fc in range(2):
                    yield lambda rt4=rt4, fc=fc: op_mm(rt4, fc)

        def chunk_fillers(pc, ops):
            """Interleave-friendly filler order: out-proj units separate the
            three proj accumulations and the transposes, so consecutive
            work-pool allocations stay well apart in time."""
            units = []
            o = list(ops)
            if pc is not None:
                xt = xts.pop(pc)
                for w in range(3):
                    if o:
                        units.append(o.pop(0))
                    units.extend(proj_accum_units(pc, xt, w))
                for t in t_units(pc):
                    if o:
                        units.append(o.pop(0))
                    units.append(t)
            units.extend(o)
            return units

        def attn_chunk(b, j, fillers):
            """Attention for query chunk (b,j) with fillers interleaved."""
            t0 = j * 512
            nv = 4 * j + 4
            cn = cpool.tile([P, 512], io_dt, tag="cn", name="cn")
            cps = [psC.tile([65, 512], F32, tag="c", name=f"cps{h}")
                   for h in range(2)]
            nf = len(fillers)
            fi = 0
            resv = min(6, nf)
            navail = nf - resv
            pend = [None]

            def issue_scores(i):
                delta = i * P - t0
                col0 = max(0, delta)
                sg = i * P
                bias = mskb_sb[:, b * NT + i:b * NT + i + 1]
                pts = []
                for h in range(2):
                    hp = slice(h * 64, (h + 1) * 64)
                    sp = psS.tile([P, 512], F32, tag="s", name="sp")
                    nc.tensor.matmul(
                        sp[:, col0:512],
                        qkt_sbs[b][hp, 1, sg:sg + P],
                        qkt_sbs[b][hp, 0, t0 + col0:t0 + 512],
                        start=True, stop=True,
                    )
                    pt = ppool.tile([P, 512], io_dt, tag="pt", name="pt")
                    nc.scalar.activation(
                        pt[:, col0:512], sp[:, col0:512], AF.Exp, bias=bias)
                    if delta >= 0:
                        # diagonal tile: zero the upper triangle of the
                        # probs on DVE (bf16 SBUF 2x) instead of a -1e4 add
                        # on the PE; the ones-column denominator then
                        # excludes the masked probs automatically
                        nc.vector.tensor_mul(
                            pt[:, col0:col0 + P], pt[:, col0:col0 + P],
                            tri_sb[:])
                    pts.append(pt)
                return (pts, col0, i)

            def issue_ctx(pts, col0, i):
                for h in range(2):
                    nc.tensor.matmul(
                        cps[h][:, col0:512],
                        v_sbs[b][:, i, h],
                        pts[h][:, col0:512],
                        start=(i == 0), stop=(i == nv - 1),
                    )

            for i in range(nv):
                hi = (navail * (i + 1)) // nv
                if fi < hi:
                    fillers[fi]()
                    fi += 1
                nxt = issue_scores(i)
                while fi < hi:
                    fillers[fi]()
                    fi += 1
                if pend[0] is not None:
                    issue_ctx(*pend[0])
                pend[0] = nxt
            issue_ctx(*pend[0])

            # normalization: denom rows -> per-head PE broadcast into one
            # work tile -> one reciprocal -> per-head scale.  Reserved
            # fillers cover the DVE->PE round trips.
            dens = []
            for h in range(2):
                den = spool.tile([1, 512], io_dt, tag="den", name="den")
                nc.vector.tensor_copy(den[:], cps[h][64:65, :])
                dens.append(den)
            while fi < nf - 2:
                fillers[fi]()
                fi += 1
            bps = psS.tile([P, 512], F32, tag="s", name="bps")
            for h in range(2):
                hp = slice(h * 64, (h + 1) * 64)
                nc.tensor.matmul(bps[hp, :], ones_sb[:, 0:64], dens[h][:],
                                 start=True, stop=True)
            while fi < nf:
                fillers[fi]()
                fi += 1
            rc = spool.tile([P, 512], F32, tag="rc", name="rc")
            nc.vector.reciprocal_approx_fast(rc[:], bps[:])
            for h in range(2):
                hp = slice(h * 64, (h + 1) * 64)
                nc.vector.tensor_mul(cn[hp, :], cps[h][0:64, :], rc[hp, :])
            return cn

        # ---- emission schedule ----
        xts = {0: emit_x_dma(0), 1: emit_x_dma(1)}
        for u in chunk_fillers(0, []):
            u()
        prev = None  # (b, j, cn) of the chunk awaiting out-projection
        for b in range(B):
            for j in range(TJ):
                pc = 4 * b + j + 1
                if pc + 1 < B * NCH:
                    xts[pc + 1] = emit_x_dma(pc + 1)
                ops = list(outproj_units(*prev)) if prev is not None else []
                fillers = chunk_fillers(pc if pc < B * NCH else None, ops)
                cn = attn_chunk(b, j, fillers)
                prev = (b, j, cn)
        for u in outproj_units(*prev):
            u()
    nc.compile()
    return nc


def make_core_inputs(x, key_padding_mask, Wqkv_w, Wqkv_b, out_w, B=4, S=2048,
                     np_io=None):
    """Host-side shard prep. Returns list of in_maps per core."""
    import ml_dtypes
    if np_io is None:
        np_io = ml_dtypes.bfloat16
    E = 1024
    P = 128
    NT = S // P
    x = np.asarray(x, np.float32)
    mask = np.asarray(key_padding_mask)
    Wqkv_w = np.asarray(Wqkv_w, np.float32)
    Wqkv_b = np.asarray(Wqkv_b, np.float32)
    out_w = np.asarray(out_w, np.float32)

    xT = np.ascontiguousarray(x.reshape(B * S, E).T).astype(np_io)
    m01 = mask.astype(np.float32)  # 1 valid / 0 padded
    mskb = np.where(m01 > 0.5, 0.0, NEG).astype(np.float32)
    mskb_t = np.ascontiguousarray(mskb.reshape(B * NT, P).T)  # [128, B*NT]
    r = np.arange(P)
    tri01 = (r[:, None] <= r[None, :]).astype(np.float32).astype(np_io)
    idn = np.eye(P, dtype=np.float32).astype(np_io)
    scale = 1.0 / np.sqrt(64.0)

    in_maps = []
    for c in range(N_CORES):
        hA, hB = 2 * c, 2 * c + 1
        sel = np.r_[hA * 64:(hA + 1) * 64, hB * 64:(hB + 1) * 64]
        wq = np.ascontiguousarray(Wqkv_w[sel].T).astype(np_io)
        wk = np.ascontiguousarray((Wqkv_w[E + sel] * scale).T).astype(np_io)
        wv = np.ascontiguousarray(Wqkv_w[2 * E + sel].T).astype(np_io)
        bq = np.ascontiguousarray(Wqkv_b[sel][:, None]).astype(np.float32)
        bv = np.ascontiguousarray(Wqkv_b[2 * E + sel][:, None]).astype(np.float32)
        ow = np.ascontiguousarray(out_w[:, sel].T).astype(np_io)
        in_maps.append({
            "xT": xT, "wq": wq, "wk": wk, "wv": wv,
            "bq": bq, "bv": bv, "ow": ow, "mskb": mskb_t,
            "tri01": tri01, "idn": idn,
        })
    return in_maps


_NC_CACHE = {}


def _get_nc(B=4, S=2048, io_dt=BF16):
    key = (B, S, io_dt)
    if key not in _NC_CACHE:
        _NC_CACHE[key] = build_program(B, S, io_dt)
    return _NC_CACHE[key]


def run_full(inputs, trace=False, tmpdir=None, io_dt=BF16, np_io=None):
    from concourse.bass_utils import run_bass_kernel_spmd

    B, S, E = 4, 2048, 1024
    nc = _get_nc(B, S, io_dt)
    in_maps = make_core_inputs(
        inputs["x"], inputs["key_padding_mask"], inputs["Wqkv_w"],
        inputs["Wqkv_b"], inputs["out_w"], B, S, np_io=np_io,
    )
    res = run_bass_kernel_spmd(
        nc, in_maps, list(range(N_CORES)), trace=trace, tmpdir=tmpdir,
    )
    acc = res.results[0]["outp"].astype(np.float32)
    for c in range(1, N_CORES):
        acc = acc + res.results[c]["outp"].astype(np.float32)
    out = acc + np.asarray(inputs["out_b"], np.float32)[None, :]
    return out.reshape(B, S, E), res


def kernel(**inputs) -> np.ndarray:
    out, _ = run_full(inputs)
    return out


# revision 23
# speedup vs baseline: 1.0449x; 1.0398x over previous
"""Trainium2 Bass kernel for causal MHA block (b=4, s=2048, E=1024, 16 heads).

Sharding: tensor-parallel over heads — 2 heads per core across 8 cores.
Each core computes Q^T/K^T (transposed layout, head-packed), V (natural
layout, ones-augmented), block-causal attention with softmax denominators
obtained for free from the ones column, and a partial out-projection over
its 128 embedding dims. Host sums the 8 partials and adds out_b.

v3 scheduling (the math is the baseline's; the issue order isn't):
  - software-pipelined attention: scores+exp for key-tile i+1 issue before
    the ctx matmuls of tile i, so PE never blocks on the ACT round trip.
  - proj/out-proj matmuls interleave as fillers BETWEEN attention
    iterations (deadline-JIT: proj chunk 4b+j+1 runs inside attn chunk
    (b,j)), keeping PE continuously busy (no HAM re-throttle).
  - key-padding mask folded into the exp bias (per-partition = per-key
    0/-10000), so V needs no masking and the transpose drain is a plain
    copy; the ones-column denominator stays exact (padded probs are 0).
  - per-head 1-bank score tiles, bufs=3: deeper rotation than one 2-bank
    tile pair, and frees a PSUM bank for the work pool.
  - PSUM: scores [128,512]x3 + ctx [65,512]x2 + work [128,512]x3 = 8 banks.
  - all PSUM->SBUF staging on DVE; exp exclusively on ACT; filler order
    keeps work-pool allocations >= 3 apart in time.
"""

import sys
from contextlib import ExitStack

import numpy as np

sys.path.insert(0, "/opt/trn_rl_repo")

import concourse.bass as bass  # noqa: E402
import concourse.tile as tile  # noqa: E402
from concourse import bacc  # noqa: E402
from concourse import mybir  # noqa: E402

F32 = mybir.dt.float32
BF16 = mybir.dt.bfloat16
AF = mybir.ActivationFunctionType

NEG = -10000.0
N_CORES = 8


def build_program(B=4, S=2048, io_dt=BF16):
    P = 128
    E = 1024
    ET = E // P            # 8 E-tiles
    RC = 512               # row chunk for projections
    NCH = S // RC          # proj chunks per batch (4)
    NT = S // P            # s-tiles per batch (16)
    TJ = S // 512          # query chunks of 512 per batch (4)
    ROWS = B * S

    nc = bacc.Bacc("TRN2", target_bir_lowering=False, debug=False)

    xT_d = nc.declare_dram_parameter("xT", [E, ROWS], io_dt, isOutput=False)
    wq_d = nc.declare_dram_parameter("wq", [E, P], io_dt, isOutput=False)
    wk_d = nc.declare_dram_parameter("wk", [E, P], io_dt, isOutput=False)
    wv_d = nc.declare_dram_parameter("wv", [E, P], io_dt, isOutput=False)
    bq_d = nc.declare_dram_parameter("bq", [P, 1], F32, isOutput=False)
    bv_d = nc.declare_dram_parameter("bv", [P, 1], F32, isOutput=False)
    ow_d = nc.declare_dram_parameter("ow", [P, E], io_dt, isOutput=False)
    mskb_d = nc.declare_dram_parameter("mskb", [P, B * NT], F32, isOutput=False)
    tri_d = nc.declare_dram_parameter("tri01", [P, P], io_dt, isOutput=False)
    idn_d = nc.declare_dram_parameter("idn", [P, P], io_dt, isOutput=False)
    out_d = nc.declare_dram_parameter("outp", [ROWS, E], io_dt, isOutput=True)

    with ExitStack() as ctx:
        tc = ctx.enter_context(tile.TileContext(nc))
        const = ctx.enter_context(tc.tile_pool(name="const", bufs=1))

        wq_sb = [const.tile([P, P], io_dt, tag=f"wq{et}", name=f"wq{et}")
                 for et in range(ET)]
        wk_sb = [const.tile([P, P], io_dt, tag=f"wk{et}", name=f"wk{et}")
                 for et in range(ET)]
        wv_sb = [const.tile([P, P], io_dt, tag=f"wv{et}", name=f"wv{et}")
                 for et in range(ET)]
        for et in range(ET):
            esl = slice(et * P, (et + 1) * P)
            nc.gpsimd.dma_start(wq_sb[et][:], wq_d[esl, :])
            nc.scalar.dma_start(wk_sb[et][:], wk_d[esl, :])
            (nc.gpsimd if et % 2 else nc.scalar).dma_start(wv_sb[et][:], wv_d[esl, :])
        ow_sb = const.tile([P, E], io_dt, tag="ow")
        nc.scalar.dma_start(ow_sb[:], ow_d[:])
        bq_sb = const.tile([P, 1], F32, tag="bq")
        nc.gpsimd.dma_start(bq_sb[:], bq_d[:])
        bv_sb = const.tile([P, 1], F32, tag="bv")
        nc.gpsimd.dma_start(bv_sb[:], bv_d[:])
        mskb_sb = const.tile([P, B * NT], F32, tag="mskb")
        nc.gpsimd.dma_start(mskb_sb[:], mskb_d[:])
        tri_sb = const.tile([P, P], io_dt, tag="tri")
        nc.gpsimd.dma_start(tri_sb[:], tri_d[:])
        idn_sb = const.tile([P, P], io_dt, tag="idn")
        nc.gpsimd.dma_start(idn_sb[:], idn_d[:])
        ones_sb = const.tile([1, P], io_dt, tag="ones")
        nc.any.memset(ones_sb[:], 1.0)

        # per-batch projection outputs: q/k packed in one tile
        qkt_sbs = [const.tile([P, 2, S], io_dt, tag=f"qk{b}", name=f"qk{b}")
                   for b in range(B)]
        v_sbs = [const.tile([P, NT, 2, 65], io_dt, tag=f"v{b}", name=f"v{b}")
                 for b in range(B)]
        # ones-augmentation columns: plain 1.0 (padded keys are zeroed via
        # the exp bias, so probs — and hence the denominator — are exact)
        for b in range(B):
            for h in range(2):
                nc.vector.memset(v_sbs[b][:, :, h, 64:65], 1.0)

        xpool = ctx.enter_context(tc.tile_pool(name="xp", bufs=2))
        ppool = ctx.enter_context(tc.tile_pool(name="pt", bufs=6))
        cpool = ctx.enter_context(tc.tile_pool(name="cn", bufs=2))
        spool = ctx.enter_context(tc.tile_pool(name="sm", bufs=2))
        opool = ctx.enter_context(tc.tile_pool(name="ot", bufs=4))
        # PSUM: "s" scores [128,512] x3; "c" ctx accum x2; "w" work x3 = 8
        psS = ctx.enter_context(tc.tile_pool(name="psS", bufs=3, space="PSUM"))
        psC = ctx.enter_context(tc.tile_pool(name="psC", bufs=2, space="PSUM"))
        psW = ctx.enter_context(tc.tile_pool(name="psW", bufs=3, space="PSUM"))

        def emit_x_dma(pc):
            """Prefetch x tiles for proj chunk pc (global index)."""
            b, ch = divmod(pc, NCH)
            r0 = b * S + ch * RC
            xt = xpool.tile([P, ET, RC], io_dt, tag="xt", name="xt")
            for et in range(ET):
                nc.sync.dma_start(xt[:, et], xT_d[et * P:(et + 1) * P, r0:r0 + RC])
            return xt

        def proj_accum_units(pc, xt, w):
            """Filler closures for one projection (w: 0=q 1=k 2=v) of proj
            chunk pc — a single work-pool accumulation over 8 matmuls."""
            b, ch = divmod(pc, NCH)
            rsb = slice(ch * RC, (ch + 1) * RC)
            acc = [None]

            def p_mm(et):
                if et == 0:
                    acc[0] = psW.tile([P, RC], F32, tag="w", name="pps")
                w_sb = (wq_sb, wk_sb, wv_sb)[w]
                nc.tensor.matmul(
                    acc[0][:], w_sb[et][:], xt[:, et],
                    start=(et == 0), stop=(et == ET - 1),
                )
                if et == ET - 1:
                    if w == 0:
                        nc.vector.tensor_scalar_add(
                            qkt_sbs[b][:, 0, rsb], acc[0][:], bq_sb[:])
                    elif w == 1:
                        # K bias cancels in softmax; plain downcast copy
                        nc.vector.tensor_copy(qkt_sbs[b][:, 1, rsb], acc[0][:])
                    else:
                        vt = xpool.tile([P, RC], io_dt, tag="vt", name="vt")
                        nc.vector.tensor_scalar_add(
                            vt[:], acc[0][:], bv_sb[:])
                        vts[pc] = vt

            for et in range(ET):
                yield lambda et=et: p_mm(et)

        vts = {}

        def t_units(pc):
            """V-transpose fillers (one per 128-key tile of proj chunk pc)."""
            b, ch = divmod(pc, NCH)

            def t_mm(rt4):
                rt = ch * (RC // P) + rt4
                trp = psW.tile([P, 2, 64], io_dt, tag="w", name="trp")
                nc.tensor.transpose(
                    trp[:], vts[pc][:, rt4 * P:(rt4 + 1) * P], idn_sb[:])
                nc.vector.tensor_copy(v_sbs[b][:, rt, :, 0:64], trp[:])

            for rt4 in range(RC // P):
                yield lambda rt4=rt4: t_mm(rt4)

        def outproj_units(b, j, cn):
            """Out-projection fillers for chunk (b,j)."""
            t0 = j * 512

            def op_mm(rt4, fc):
                r0 = b * S + t0 + rt4 * P
                ops = psW.tile([P, 512], F32, tag="w", name="ops")
                nc.tensor.matmul(
                    ops[:],
                    cn[:, rt4 * P:(rt4 + 1) * P],
                    ow_sb[:, fc * 512:(fc + 1) * 512],
                    start=True, stop=True,
                )
                ot = opool.tile([P, 512], io_dt, tag="ot", name="ot")
                nc.vector.tensor_copy(ot[:], ops[:])
                nc.sync.dma_start(
                    out_d[r0:r0 + P, fc * 512:(fc + 1) * 512], ot[:])

            for rt4 in range(4):
                for fc in range(2):
                    yield lambda rt4=rt4, fc=fc: op_mm(rt4, fc)

        def chunk_fillers(pc, ops):
            """Interleave-friendly filler order: out-proj units separate the
            three proj accumulations and the transposes, so consecutive
            work-pool allocations stay well apart in time."""
            units = []
            o = list(ops)
            if pc is not None:
                xt = xts.pop(pc)
                for w in range(3):
                    if o:
                        units.append(o.pop(0))
                    units.extend(proj_accum_units(pc, xt, w))
                for t in t_units(pc):
                    if o:
                        units.append(o.pop(0))
                    units.append(t)
            units.extend(o)
            return units

        def attn_chunk(b, j, fillers):
            """Attention for query chunk (b,j) with fillers interleaved."""
            t0 = j * 512
            nv = 4 * j + 4
            cn = cpool.tile([P, 512], io_dt, tag="cn", name="cn")
            cps = [psC.tile([65, 512], F32, tag="c", name=f"cps{h}")
                   for h in range(2)]
            nf = len(fillers)
            fi = 0
            resv = min(6, nf)
            navail = nf - resv
            pend = [None]

            def issue_scores(i):
                delta = i * P - t0
                col0 = max(0, delta)
                sg = i * P
                bias = mskb_sb[:, b * NT + i:b * NT + i + 1]
                pts = []
                for h in range(2):
                    hp = slice(h * 64, (h + 1) * 64)
                    sp = psS.tile([P, 512], F32, tag="s", name="sp")
                    nc.tensor.matmul(
                        sp[:, col0:512],
                        qkt_sbs[b][hp, 1, sg:sg + P],
                        qkt_sbs[b][hp, 0, t0 + col0:t0 + 512],
                        start=True, stop=True,
                    )
                    pt = ppool.tile([P, 512], io_dt, tag="pt", name="pt")
                    nc.scalar.activation(
                        pt[:, col0:512], sp[:, col0:512], AF.Exp, bias=bias)
                    if delta >= 0:
                        # diagonal tile: zero the upper triangle of the
                        # probs on DVE (bf16 SBUF 2x) instead of a -1e4 add
                        # on the PE; the ones-column denominator then
                        # excludes the masked probs automatically
                        nc.vector.tensor_mul(
                            pt[:, col0:col0 + P], pt[:, col0:col0 + P],
                            tri_sb[:])
                    pts.append(pt)
                return (pts, col0, i)

            def issue_ctx(pts, col0, i):
                for h in range(2):
                    nc.tensor.matmul(
                        cps[h][:, col0:512],
                        v_sbs[b][:, i, h],
                        pts[h][:, col0:512],
                        start=(i == nv - 1), stop=(i == 0),
                    )

            # key tiles in DESCENDING order: the narrow diagonal iterations
            # run first (when the filler supply is largest) and the wide
            # full-512 iterations run last, so the chunk tail keeps the PE
            # array duty high and HAM never re-throttles at the boundary
            for k, i in enumerate(range(nv - 1, -1, -1)):
                hi = (navail * (k + 1)) // nv
                if fi < hi:
                    fillers[fi]()
                    fi += 1
                nxt = issue_scores(i)
                while fi < hi:
                    fillers[fi]()
                    fi += 1
                if pend[0] is not None:
                    issue_ctx(*pend[0])
                pend[0] = nxt
            issue_ctx(*pend[0])

            # normalization: denom rows -> per-head PE broadcast into one
            # work tile -> one reciprocal -> per-head scale.  Reserved
            # fillers cover the DVE->PE round trips.
            dens = []
            for h in range(2):
                den = spool.tile([1, 512], io_dt, tag="den", name="den")
                nc.vector.tensor_copy(den[:], cps[h][64:65, :])
                dens.append(den)
            while fi < nf - 2:
                fillers[fi]()
                fi += 1
            bps = psS.tile([P, 512], F32, tag="s", name="bps")
            for h in range(2):
                hp = slice(h * 64, (h + 1) * 64)
                nc.tensor.matmul(bps[hp, :], ones_sb[:, 0:64], dens[h][:],
                                 start=True, stop=True)
            while fi < nf:
                fillers[fi]()
                fi += 1
            rc = spool.tile([P, 512], F32, tag="rc", name="rc")
            nc.vector.reciprocal_approx_fast(rc[:], bps[:])
            for h in range(2):
                hp = slice(h * 64, (h + 1) * 64)
                nc.vector.tensor_mul(cn[hp, :], cps[h][0:64, :], rc[hp, :])
            return cn

        # ---- emission schedule ----
        xts = {0: emit_x_dma(0), 1: emit_x_dma(1)}
        for u in chunk_fillers(0, []):
            u()
        prev = None  # (b, j, cn) of the chunk awaiting out-projection
        for b in range(B):
            for j in range(TJ):
                pc = 4 * b + j + 1
                if pc + 1 < B * NCH:
                    xts[pc + 1] = emit_x_dma(pc + 1)
                ops = list(outproj_units(*prev)) if prev is not None else []
                fillers = chunk_fillers(pc if pc < B * NCH else None, ops)
                cn = attn_chunk(b, j, fillers)
                prev = (b, j, cn)
        for u in outproj_units(*prev):
            u()
    nc.compile()
    return nc


def make_core_inputs(x, key_padding_mask, Wqkv_w, Wqkv_b, out_w, B=4, S=2048,
                     np_io=None):
    """Host-side shard prep. Returns list of in_maps per core."""
    import ml_dtypes
    if np_io is None:
        np_io = ml_dtypes.bfloat16
    E = 1024
    P = 128
    NT = S // P
    x = np.asarray(x, np.float32)
    mask = np.asarray(key_padding_mask)
    Wqkv_w = np.asarray(Wqkv_w, np.float32)
    Wqkv_b = np.asarray(Wqkv_b, np.float32)
    out_w = np.asarray(out_w, np.float32)

    xT = np.ascontiguousarray(x.reshape(B * S, E).T).astype(np_io)
    m01 = mask.astype(np.float32)  # 1 valid / 0 padded
    mskb = np.where(m01 > 0.5, 0.0, NEG).astype(np.float32)
    mskb_t = np.ascontiguousarray(mskb.reshape(B * NT, P).T)  # [128, B*NT]
    r = np.arange(P)
    tri01 = (r[:, None] <= r[None, :]).astype(np.float32).astype(np_io)
    idn = np.eye(P, dtype=np.float32).astype(np_io)
    scale = 1.0 / np.sqrt(64.0)

    in_maps = []
    for c in range(N_CORES):
        hA, hB = 2 * c, 2 * c + 1
        sel = np.r_[hA * 64:(hA + 1) * 64, hB * 64:(hB + 1) * 64]
        wq = np.ascontiguousarray(Wqkv_w[sel].T).astype(np_io)
        wk = np.ascontiguousarray((Wqkv_w[E + sel] * scale).T).astype(np_io)
        wv = np.ascontiguousarray(Wqkv_w[2 * E + sel].T).astype(np_io)
        bq = np.ascontiguousarray(Wqkv_b[sel][:, None]).astype(np.float32)
        bv = np.ascontiguousarray(Wqkv_b[2 * E + sel][:, None]).astype(np.float32)
        ow = np.ascontiguousarray(out_w[:, sel].T).astype(np_io)
        in_maps.append({
            "xT": xT, "wq": wq, "wk": wk, "wv": wv,
            "bq": bq, "bv": bv, "ow": ow, "mskb": mskb_t,
            "tri01": tri01, "idn": idn,
        })
    return in_maps


_NC_CACHE = {}


def _get_nc(B=4, S=2048, io_dt=BF16):
    key = (B, S, io_dt)
    if key not in _NC_CACHE:
        _NC_CACHE[key] = build_program(B, S, io_dt)
    return _NC_CACHE[key]


def run_full(inputs, trace=False, tmpdir=None, io_dt=BF16, np_io=None):
    from concourse.bass_utils import run_bass_kernel_spmd

    B, S, E = 4, 2048, 1024
    nc = _get_nc(B, S, io_dt)
    in_maps = make_core_inputs(
        inputs["x"], inputs["key_padding_mask"], inputs["Wqkv_w"],
        inputs["Wqkv_b"], inputs["out_w"], B, S, np_io=np_io,
    )
    res = run_bass_kernel_spmd(
        nc, in_maps, list(range(N_CORES)), trace=trace, tmpdir=tmpdir,
    )
    acc = res.results[0]["outp"].astype(np.float32)
    for c in range(1, N_CORES):
        acc = acc + res.results[c]["outp"].astype(np.float32)
    out = acc + np.asarray(inputs["out_b"], np.float32)[None, :]
    return out.reshape(B, S, E), res


def kernel(**inputs) -> np.ndarray:
    out, _ = run_full(inputs)
    return out


# revision 27
# speedup vs baseline: 1.0828x; 1.0363x over previous
"""Trainium2 Bass kernel for causal MHA block (b=4, s=2048, E=1024, 16 heads).

Sharding: tensor-parallel over heads — 2 heads per core across 8 cores.
Each core computes Q^T/K^T (transposed layout, head-packed), V (natural
layout, ones-augmented), block-causal attention with softmax denominators
obtained for free from the ones column, and a partial out-projection over
its 128 embedding dims. Host sums the 8 partials and adds out_b.

v3 scheduling (the math is the baseline's; the issue order isn't):
  - software-pipelined attention: scores+exp for key-tile i+1 issue before
    the ctx matmuls of tile i, so PE never blocks on the ACT round trip.
  - proj/out-proj matmuls interleave as fillers BETWEEN attention
    iterations (deadline-JIT: proj chunk 4b+j+1 runs inside attn chunk
    (b,j)), keeping PE continuously busy (no HAM re-throttle).
  - key-padding mask folded into the exp bias (per-partition = per-key
    0/-10000), so V needs no masking and the transpose drain is a plain
    copy; the ones-column denominator stays exact (padded probs are 0).
  - per-head 1-bank score tiles, bufs=3: deeper rotation than one 2-bank
    tile pair, and frees a PSUM bank for the work pool.
  - PSUM: scores [128,512]x3 + ctx [65,512]x2 + work [128,512]x3 = 8 banks.
  - all PSUM->SBUF staging on DVE; exp exclusively on ACT; filler order
    keeps work-pool allocations >= 3 apart in time.
"""

import sys
from contextlib import ExitStack

import numpy as np

sys.path.insert(0, "/opt/trn_rl_repo")

import concourse.bass as bass  # noqa: E402
import concourse.tile as tile  # noqa: E402
from concourse import bacc  # noqa: E402
from concourse import mybir  # noqa: E402

F32 = mybir.dt.float32
BF16 = mybir.dt.bfloat16
AF = mybir.ActivationFunctionType

NEG = -10000.0
N_CORES = 8


def build_program(B=4, S=2048, io_dt=BF16):
    P = 128
    E = 1024
    ET = E // P            # 8 E-tiles
    RC = 512               # row chunk for projections
    NCH = S // RC          # proj chunks per batch (4)
    NT = S // P            # s-tiles per batch (16)
    TJ = S // 512          # query chunks of 512 per batch (4)
    ROWS = B * S

    nc = bacc.Bacc("TRN2", target_bir_lowering=False, debug=False)

    xT_d = nc.declare_dram_parameter("xT", [E, ROWS], io_dt, isOutput=False)
    wq_d = nc.declare_dram_parameter("wq", [E, P], io_dt, isOutput=False)
    wk_d = nc.declare_dram_parameter("wk", [E, P], io_dt, isOutput=False)
    wv_d = nc.declare_dram_parameter("wv", [E, P], io_dt, isOutput=False)
    bq_d = nc.declare_dram_parameter("bq", [P, 1], F32, isOutput=False)
    bv_d = nc.declare_dram_parameter("bv", [P, 1], F32, isOutput=False)
    ow_d = nc.declare_dram_parameter("ow", [P, E], io_dt, isOutput=False)
    mskb_d = nc.declare_dram_parameter("mskb", [P, B * NT], F32, isOutput=False)
    tri_d = nc.declare_dram_parameter("tri01", [P, P], io_dt, isOutput=False)
    idn_d = nc.declare_dram_parameter("idn", [P, P], io_dt, isOutput=False)
    out_d = nc.declare_dram_parameter("outp", [ROWS, E], io_dt, isOutput=True)

    with ExitStack() as ctx:
        tc = ctx.enter_context(tile.TileContext(nc))
        const = ctx.enter_context(tc.tile_pool(name="const", bufs=1))

        wq_sb = [const.tile([P, P], io_dt, tag=f"wq{et}", name=f"wq{et}")
                 for et in range(ET)]
        wk_sb = [const.tile([P, P], io_dt, tag=f"wk{et}", name=f"wk{et}")
                 for et in range(ET)]
        wv_sb = [const.tile([P, P], io_dt, tag=f"wv{et}", name=f"wv{et}")
                 for et in range(ET)]
        for et in range(ET):
            esl = slice(et * P, (et + 1) * P)
            nc.gpsimd.dma_start(wq_sb[et][:], wq_d[esl, :])
            nc.scalar.dma_start(wk_sb[et][:], wk_d[esl, :])
            (nc.gpsimd if et % 2 else nc.scalar).dma_start(wv_sb[et][:], wv_d[esl, :])
        ow_sb = const.tile([P, E], io_dt, tag="ow")
        nc.scalar.dma_start(ow_sb[:], ow_d[:])
        bq_sb = const.tile([P, 1], F32, tag="bq")
        nc.gpsimd.dma_start(bq_sb[:], bq_d[:])
        bv_sb = const.tile([P, 1], F32, tag="bv")
        nc.gpsimd.dma_start(bv_sb[:], bv_d[:])
        mskb_sb = const.tile([P, B * NT], F32, tag="mskb")
        nc.gpsimd.dma_start(mskb_sb[:], mskb_d[:])
        tri_sb = const.tile([P, P], io_dt, tag="tri")
        nc.gpsimd.dma_start(tri_sb[:], tri_d[:])
        idn_sb = const.tile([P, P], io_dt, tag="idn")
        nc.gpsimd.dma_start(idn_sb[:], idn_d[:])
        ones_sb = const.tile([1, P], io_dt, tag="ones")
        nc.any.memset(ones_sb[:], 1.0)

        # per-batch projection outputs: q/k packed in one tile
        qkt_sbs = [const.tile([P, 2, S], io_dt, tag=f"qk{b}", name=f"qk{b}")
                   for b in range(B)]
        v_sbs = [const.tile([P, NT, 2, 65], io_dt, tag=f"v{b}", name=f"v{b}")
                 for b in range(B)]
        # ones-augmentation columns: plain 1.0 (padded keys are zeroed via
        # the exp bias, so probs — and hence the denominator — are exact)
        for b in range(B):
            for h in range(2):
                nc.vector.memset(v_sbs[b][:, :, h, 64:65], 1.0)

        xpool = ctx.enter_context(tc.tile_pool(name="xp", bufs=2))
        ppool = ctx.enter_context(tc.tile_pool(name="pt", bufs=6))
        cpool = ctx.enter_context(tc.tile_pool(name="cn", bufs=2))
        spool = ctx.enter_context(tc.tile_pool(name="sm", bufs=2))
        opool = ctx.enter_context(tc.tile_pool(name="ot", bufs=4))
        # PSUM: "s" scores [128,512] x3; "c" ctx accum x2; "w" work x3 = 8
        psS = ctx.enter_context(tc.tile_pool(name="psS", bufs=3, space="PSUM"))
        psC = ctx.enter_context(tc.tile_pool(name="psC", bufs=2, space="PSUM"))
        psW = ctx.enter_context(tc.tile_pool(name="psW", bufs=3, space="PSUM"))

        def emit_x_dma(pc):
            """Prefetch x tiles for proj chunk pc (global index)."""
            b, ch = divmod(pc, NCH)
            r0 = b * S + ch * RC
            xt = xpool.tile([P, ET, RC], io_dt, tag="xt", name="xt")
            for et in range(ET):
                nc.sync.dma_start(xt[:, et], xT_d[et * P:(et + 1) * P, r0:r0 + RC])
            return xt

        def proj_accum_units(pc, xt, w):
            """Filler closures for one projection (w: 0=q 1=k 2=v) of proj
            chunk pc — a single work-pool accumulation over 8 matmuls."""
            b, ch = divmod(pc, NCH)
            rsb = slice(ch * RC, (ch + 1) * RC)
            acc = [None]

            def p_mm(et):
                if et == 0:
                    acc[0] = psW.tile([P, RC], F32, tag="w", name="pps")
                w_sb = (wq_sb, wk_sb, wv_sb)[w]
                nc.tensor.matmul(
                    acc[0][:], w_sb[et][:], xt[:, et],
                    start=(et == 0), stop=(et == ET - 1),
                )
                if et == ET - 1:
                    if w == 0:
                        nc.vector.tensor_scalar_add(
                            qkt_sbs[b][:, 0, rsb], acc[0][:], bq_sb[:])
                    elif w == 1:
                        # K bias cancels in softmax; plain downcast copy
                        nc.vector.tensor_copy(qkt_sbs[b][:, 1, rsb], acc[0][:])
                    else:
                        vt = xpool.tile([P, RC], io_dt, tag="vt", name="vt")
                        nc.vector.tensor_scalar_add(
                            vt[:], acc[0][:], bv_sb[:])
                        vts[pc] = vt

            for et in range(ET):
                yield lambda et=et: p_mm(et)

        vts = {}

        def t_units(pc):
            """V-transpose fillers (one per 128-key tile of proj chunk pc)."""
            b, ch = divmod(pc, NCH)

            def t_mm(rt4):
                rt = ch * (RC // P) + rt4
                trp = psW.tile([P, 2, 64], io_dt, tag="w", name="trp")
                nc.tensor.transpose(
                    trp[:], vts[pc][:, rt4 * P:(rt4 + 1) * P], idn_sb[:])
                nc.vector.tensor_copy(v_sbs[b][:, rt, :, 0:64], trp[:])

            for rt4 in range(RC // P):
                yield lambda rt4=rt4: t_mm(rt4)

        def outproj_units(b, j, cn):
            """Out-projection fillers for chunk (b,j)."""
            t0 = j * 512

            def op_mm(rt4, fc):
                r0 = b * S + t0 + rt4 * P
                ops = psW.tile([P, 512], F32, tag="w", name="ops")
                nc.tensor.matmul(
                    ops[:],
                    cn[:, rt4 * P:(rt4 + 1) * P],
                    ow_sb[:, fc * 512:(fc + 1) * 512],
                    start=True, stop=True,
                )
                ot = opool.tile([P, 512], io_dt, tag="ot", name="ot")
                nc.vector.tensor_copy(ot[:], ops[:])
                nc.sync.dma_start(
                    out_d[r0:r0 + P, fc * 512:(fc + 1) * 512], ot[:])

            for rt4 in range(4):
                for fc in range(2):
                    yield lambda rt4=rt4, fc=fc: op_mm(rt4, fc)

        def chunk_fillers(pc, ops):
            """Interleave-friendly filler order: out-proj units separate the
            three proj accumulations and the transposes, so consecutive
            work-pool allocations stay well apart in time."""
            units = []
            o = list(ops)
            if pc is not None:
                xt = xts.pop(pc)
                # proj first: out-proj units depend on the previous chunk's
                # normalization, which is still draining at chunk start
                for w in range(3):
                    units.extend(proj_accum_units(pc, xt, w))
                    if o:
                        units.append(o.pop(0))
                for t in t_units(pc):
                    units.append(t)
                    if o:
                        units.append(o.pop(0))
            units.extend(o)
            return units

        def attn_chunk(b, j, fillers):
            """Attention for query chunk (b,j) with fillers interleaved."""
            t0 = j * 512
            nv = 4 * j + 4
            cn = cpool.tile([P, 512], io_dt, tag="cn", name="cn")
            cps = [psC.tile([65, 512], F32, tag="c", name=f"cps{h}")
                   for h in range(2)]
            nf = len(fillers)
            fi = 0
            resv = min(8, nf)
            navail = nf - resv
            pend = [None]
            # deficit-shaped filler quotas: the first (narrow, ACT-overhead-
            # dominated) iterations and the region right after the previous
            # chunk's normalization need the most PE filler work
            quota = [2 if k < 4 else 1 for k in range(nv)]
            extra = navail - sum(quota)
            k = 0
            while extra > 0:
                quota[k % nv] += 1
                extra -= 1
                k += 1
            while extra < 0:
                k -= 1
                if quota[k % nv] > 0:
                    quota[k % nv] -= 1
                    extra += 1
            cum = []
            s = 0
            for q in quota:
                s += q
                cum.append(min(s, navail))

            def issue_scores(i):
                delta = i * P - t0
                col0 = max(0, delta)
                sg = i * P
                bias = mskb_sb[:, b * NT + i:b * NT + i + 1]
                pts = []
                for h in range(2):
                    hp = slice(h * 64, (h + 1) * 64)
                    sp = psS.tile([P, 512], F32, tag="s", name="sp")
                    nc.tensor.matmul(
                        sp[:, col0:512],
                        qkt_sbs[b][hp, 1, sg:sg + P],
                        qkt_sbs[b][hp, 0, t0 + col0:t0 + 512],
                        start=True, stop=True,
                    )
                    pt = ppool.tile([P, 512], io_dt, tag="pt", name="pt")
                    nc.scalar.activation(
                        pt[:, col0:512], sp[:, col0:512], AF.Exp, bias=bias)
                    if delta >= 0:
                        # diagonal tile: zero the upper triangle of the
                        # probs on DVE (bf16 SBUF 2x) instead of a -1e4 add
                        # on the PE; the ones-column denominator then
                        # excludes the masked probs automatically
                        nc.vector.tensor_mul(
                            pt[:, col0:col0 + P], pt[:, col0:col0 + P],
                            tri_sb[:])
                    pts.append(pt)
                return (pts, col0, i)

            def issue_ctx(pts, col0, i):
                for h in range(2):
                    nc.tensor.matmul(
                        cps[h][:, col0:512],
                        v_sbs[b][:, i, h],
                        pts[h][:, col0:512],
                        start=(i == nv - 1), stop=(i == 0),
                    )

            # key tiles in DESCENDING order: the narrow diagonal iterations
            # run first (when the filler supply is largest) and the wide
            # full-512 iterations run last, so the chunk tail keeps the PE
            # array duty high and HAM never re-throttles at the boundary
            for k, i in enumerate(range(nv - 1, -1, -1)):
                hi = cum[k]
                if fi < hi:
                    fillers[fi]()
                    fi += 1
                nxt = issue_scores(i)
                while fi < hi:
                    fillers[fi]()
                    fi += 1
                if pend[0] is not None:
                    issue_ctx(*pend[0])
                pend[0] = nxt
            issue_ctx(*pend[0])

            # normalization: denom rows -> per-head PE broadcast into one
            # work tile -> one reciprocal -> per-head scale.  Reserved
            # fillers cover the DVE->PE round trips.
            dens = []
            for h in range(2):
                den = spool.tile([1, 512], io_dt, tag="den", name="den")
                nc.vector.tensor_copy(den[:], cps[h][64:65, :])
                dens.append(den)
            while fi < nf - 2:
                fillers[fi]()
                fi += 1
            bps = psS.tile([P, 512], F32, tag="s", name="bps")
            for h in range(2):
                hp = slice(h * 64, (h + 1) * 64)
                nc.tensor.matmul(bps[hp, :], ones_sb[:, 0:64], dens[h][:],
                                 start=True, stop=True)
            while fi < nf:
                fillers[fi]()
                fi += 1
            rc = spool.tile([P, 512], F32, tag="rc", name="rc")
            nc.vector.reciprocal_approx_fast(rc[:], bps[:])
            for h in range(2):
                hp = slice(h * 64, (h + 1) * 64)
                nc.vector.tensor_mul(cn[hp, :], cps[h][0:64, :], rc[hp, :])
            return cn

        # ---- emission schedule ----
        xts = {0: emit_x_dma(0), 1: emit_x_dma(1)}
        for u in chunk_fillers(0, []):
            u()
        prev = None  # (b, j, cn) of the chunk awaiting out-projection
        for b in range(B):
            for j in range(TJ):
                pc = 4 * b + j + 1
                if pc + 1 < B * NCH:
                    xts[pc + 1] = emit_x_dma(pc + 1)
                ops = list(outproj_units(*prev)) if prev is not None else []
                fillers = chunk_fillers(pc if pc < B * NCH else None, ops)
                cn = attn_chunk(b, j, fillers)
                prev = (b, j, cn)
        for u in outproj_units(*prev):
            u()
    nc.compile()
    return nc


def make_core_inputs(x, key_padding_mask, Wqkv_w, Wqkv_b, out_w, B=4, S=2048,
                     np_io=None):
    """Host-side shard prep. Returns list of in_maps per core."""
    import ml_dtypes
    if np_io is None:
        np_io = ml_dtypes.bfloat16
    E = 1024
    P = 128
    NT = S // P
    x = np.asarray(x, np.float32)
    mask = np.asarray(key_padding_mask)
    Wqkv_w = np.asarray(Wqkv_w, np.float32)
    Wqkv_b = np.asarray(Wqkv_b, np.float32)
    out_w = np.asarray(out_w, np.float32)

    xT = np.ascontiguousarray(x.reshape(B * S, E).T).astype(np_io)
    m01 = mask.astype(np.float32)  # 1 valid / 0 padded
    mskb = np.where(m01 > 0.5, 0.0, NEG).astype(np.float32)
    mskb_t = np.ascontiguousarray(mskb.reshape(B * NT, P).T)  # [128, B*NT]
    r = np.arange(P)
    tri01 = (r[:, None] <= r[None, :]).astype(np.float32).astype(np_io)
    idn = np.eye(P, dtype=np.float32).astype(np_io)
    scale = 1.0 / np.sqrt(64.0)

    in_maps = []
    for c in range(N_CORES):
        hA, hB = 2 * c, 2 * c + 1
        sel = np.r_[hA * 64:(hA + 1) * 64, hB * 64:(hB + 1) * 64]
        wq = np.ascontiguousarray(Wqkv_w[sel].T).astype(np_io)
        wk = np.ascontiguousarray((Wqkv_w[E + sel] * scale).T).astype(np_io)
        wv = np.ascontiguousarray(Wqkv_w[2 * E + sel].T).astype(np_io)
        bq = np.ascontiguousarray(Wqkv_b[sel][:, None]).astype(np.float32)
        bv = np.ascontiguousarray(Wqkv_b[2 * E + sel][:, None]).astype(np.float32)
        ow = np.ascontiguousarray(out_w[:, sel].T).astype(np_io)
        in_maps.append({
            "xT": xT, "wq": wq, "wk": wk, "wv": wv,
            "bq": bq, "bv": bv, "ow": ow, "mskb": mskb_t,
            "tri01": tri01, "idn": idn,
        })
    return in_maps


_NC_CACHE = {}


def _get_nc(B=4, S=2048, io_dt=BF16):
    key = (B, S, io_dt)
    if key not in _NC_CACHE:
        _NC_CACHE[key] = build_program(B, S, io_dt)
    return _NC_CACHE[key]


def run_full(inputs, trace=False, tmpdir=None, io_dt=BF16, np_io=None):
    from concourse.bass_utils import run_bass_kernel_spmd

    B, S, E = 4, 2048, 1024
    nc = _get_nc(B, S, io_dt)
    in_maps = make_core_inputs(
        inputs["x"], inputs["key_padding_mask"], inputs["Wqkv_w"],
        inputs["Wqkv_b"], inputs["out_w"], B, S, np_io=np_io,
    )
    res = run_bass_kernel_spmd(
        nc, in_maps, list(range(N_CORES)), trace=trace, tmpdir=tmpdir,
    )
    acc = res.results[0]["outp"].astype(np.float32)
    for c in range(1, N_CORES):
        acc = acc + res.results[c]["outp"].astype(np.float32)
    out = acc + np.asarray(inputs["out_b"], np.float32)[None, :]
    return out.reshape(B, S, E), res


def kernel(**inputs) -> np.ndarray:
    out, _ = run_full(inputs)
    return out
